# revision 28
# baseline (speedup 1.0000x reference)
"""GatedAttention Trainium2 kernel.

Math (per batch b):
  Qw = x @ Wq + bq            (N, A)
  Kw = x @ Wk + bk            (N, A)
  g  = sigmoid(Qw @ Wv + bv)  (N,)
  S  = Qw @ Kw^T, diag -> -inf
  P  = softmax(S, axis=0)     (column softmax)
  out = (1-g)[:,None] * P + g[:,None] * I

Sharding: 8 cores = 4 batches x 2 column-halves of the score matrix.
Column softmax is independent per column, so no cross-core reduction for
the softmax itself.

Qw dedup: a column-shard core needs Qw for ALL 4096 rows, but each core
only PROJECTS its own 2048 rows; the other half arrives from the pair
core via two pair-wise AllGathers (DRAM bounce; they run on the
TOPSP/SDMA collective silicon and overlap compute, but back-to-back
collectives SERIALIZE, so the pipeline is sized around AG1 landing
~79us and AG2 ~115us). The AG output is rank-ordered, so each core
reads the peer slice back with an indirect row-index DMA whose index
vector is a per-core host input - the program stays pure SPMD. Gate
rows ride AG2 only: AG2's doorbell is gated by AG1's completion
anyway, while AG1's doorbell must fire the moment the first two
Q-blocks finish (~40us). This cuts PE work 944 -> 828 matmuls at
~227ns each.

Score schedule (the collective-latency-shaped part): the score loop is
emitted in arrival order so the in-order PE queue never head-blocks:
  [lo units t0-7]            75-104us  needs nothing remote
  [readback1]                          ch2 qwt casts (DVE idle then)
  [ch2 units t0-7]           104-118   needs AG1 only
  [readback2 + g1m planes]             ch3 qwt + remote gate planes
  [ch3(k), finalize(k), lo/ch2/ch3(8+k), finalize(8+k)]  k=0..7
The finalize stream (~5.9us/tile of DVE: reciprocal-scale x (1-g) stt
per 1024-quarter + diag) starts the instant AG2 lands and overlaps the
remaining PE work; exp tiles are stored as [128,1024] quarters so lo /
ch2 / ch3 chunks have independent lifetimes.

Device layout: scores computed transposed, sT[j, i] tiles (j on
partitions) so the softmax reduction over i is a free-axis reduction.
The i axis is host-permuted so each core's diagonal block sits at
i in [0, 2048).

Dtypes: x / Wq / Wk ship as fp16 (half the HBM read traffic) and are
upcast on-device to fp32r, which streams through the PE at 227ns per
512-row matmul - measurably faster than fp16/bf16 operands (259ns).
SWDGE casting DMAs are ~5x slower than plain ones, so all casts are
explicit DVE ops. Projections/scores accumulate in fp32 PSUM; Exp
output in bf16; the exchange wire format is fp16.
"""
import numpy as np

import concourse.bacc as bacc
import concourse.bass as bass
import concourse.mybir as mybir
import concourse.tile as tile
from concourse.bass_utils import run_bass_kernel_spmd

FP32 = mybir.dt.float32
FP32R = mybir.dt.float32r
FP16 = mybir.dt.float16
BF16 = mybir.dt.bfloat16
I32 = mybir.dt.int32
AF = mybir.ActivationFunctionType
ALU = mybir.AluOpType
AX = mybir.AxisListType

B, N, H, A = 4, 4096, 1024, 512
NSH = N // 2          # per-core column shard / locally projected rows
NEG = -1.0e30
AGR2 = 514            # AG2 bounce rows: 512 qwt + 2 gate rows

_CACHE = {}


def _build():
    nc = bacc.Bacc("TRN2", target_bir_lowering=False, debug=False, num_devices=8)
    xq = nc.dram_tensor("xq", [H, NSH], FP16, kind="ExternalInput").ap()
    wq = nc.dram_tensor("wq", [H, A], FP16, kind="ExternalInput").ap()
    wk = nc.dram_tensor("wk", [H, A], FP16, kind="ExternalInput").ap()
    misc = nc.dram_tensor("misc", [128, 18], FP32, kind="ExternalInput").ap()
    eye = nc.dram_tensor("eye", [128, 128], FP32, kind="ExternalInput").ap()
    idx = nc.dram_tensor("idx", [128, 9], I32, kind="ExternalInput").ap()
    out = nc.dram_tensor("out", [NSH, N], BF16, kind="ExternalOutput").ap()

    with tile.TileContext(nc) as tc:
        with (
            tc.tile_pool(name="const", bufs=1) as cpool,
            tc.tile_pool(name="proj_out", bufs=1) as qkpool,
            tc.tile_pool(name="bcast", bufs=1) as bcp,
            tc.tile_pool(name="gaterow", bufs=4) as gtmp,
            tc.tile_pool(name="xchg", bufs=4) as xch,
            tc.tile_pool(name="grpool", bufs=2) as grp,
            tc.tile_pool(name="dram", bufs=1, space="DRAM") as dram,
        ):
            # ---- memset-only constants first: the warm-up burst depends
            # only on these, so the PE starts right after the preamble.
            ones_f = cpool.tile([1, 128], FP32, tag="onesf", name="onesf")
            nc.vector.memset(ones_f[:], 1.0)
            ones_r = cpool.tile([1, 128], FP32R, tag="ones", name="ones")
            nc.vector.tensor_copy(ones_r[:], ones_f[:])
            ones_h = cpool.tile([1, 128], FP16, tag="onesh", name="onesh")
            nc.vector.tensor_copy(ones_h[:], ones_f[:])

            # ---- DMA'd constants
            ident = cpool.tile([128, 128], FP32, tag="ident", name="ident")
            nc.sync.dma_start(ident[:], eye)
            misc_sb = cpool.tile([128, 18], FP32, tag="misc", name="misc")
            nc.gpsimd.dma_start(misc_sb[:], misc)
            idx_sb = cpool.tile([128, 9], I32, tag="idx", name="idx")
            nc.gpsimd.dma_start(idx_sb[:], idx)
            identb = cpool.tile([128, 128], BF16, tag="identb", name="identb")
            nc.vector.tensor_copy(identb[:], ident[:])
            dneg = cpool.tile([128, 128], FP32, tag="dneg", name="dneg")
            nc.vector.tensor_scalar(dneg[:], ident[:], NEG, None, op0=ALU.mult)
            misc_r = cpool.tile([128, 18], FP32R, tag="miscr", name="miscr")
            nc.vector.tensor_copy(misc_r[:], misc_sb[:])

            # ---- persistent projection outputs (fp32r) ----
            qwt = [qkpool.tile([128, N], FP32R, tag=f"qwt{a}", name=f"qwt{a}")
                   for a in range(4)]
            kwt = [qkpool.tile([128, NSH], FP32R, tag=f"kwt{a}", name=f"kwt{a}")
                   for a in range(4)]
            # gate broadcast planes (bf16): g1m[p,i] = 1-g_i (all i),
            # gbc[p,i] = g_i (local i only - the diagonal never lands in the
            # remote half)
            g1m_bc = bcp.tile([128, N], BF16, tag="g1mbc", name="g1mbc")
            g_bc = bcp.tile([128, NSH], BF16, tag="gbc", name="gbc")

            # AllGather bounce buffers (DRAM). AG1: qwt cols [0:1024).
            # AG2: qwt cols [1024:2048) + rows 512/513 = g1 (=1-g) rows
            # covering local i [0:1024) / [1024:2048).
            agin = [dram.tile([512, 1024], FP16, tag="agi0", name="agi0"),
                    dram.tile([AGR2, 1024], FP16, tag="agi1", name="agi1")]
            agout = [dram.tile([1024, 1024], FP16, tag="ago0", name="ago0"),
                     dram.tile([2 * AGR2, 1024], FP16, tag="ago1",
                               name="ago1")]

            # ---- projections + fused gate + exchange ----
            with (
                tc.tile_pool(name="wtiles", bufs=1) as wpool,
                tc.tile_pool(name="wstage", bufs=3) as wst,
                tc.tile_pool(name="xstage", bufs=10) as xst,
                tc.tile_pool(name="xslices", bufs=10) as xpool,
                tc.tile_pool(name="projps", bufs=4, space="PSUM") as ppool,
                tc.tile_pool(name="zrowps", bufs=2, space="PSUM") as zpool,
                tc.tile_pool(name="bcps", bufs=2, space="PSUM") as bps,
            ):
                # PE warm-up: keeps the HAM activity monitor busy during the
                # DMA lead-in so the first real matmuls run at full clock.
                warm = ppool.tile([128, 512], FP32, tag="ps", name="warm")
                for _ in range(32):
                    nc.tensor.matmul(warm[0:64, 0:64], ones_r[:, 0:64],
                                     ones_r[:, 0:64], start=True, stop=True)

                def load_w(dram_t, h, lst, tag):
                    wt = wst.tile([128, A], FP16, tag="wst", name="wst")
                    nc.sync.dma_start(wt[:], dram_t[h * 128:(h + 1) * 128, :])
                    wr = wpool.tile([128, A], FP32R, tag=f"{tag}{h}",
                                    name=f"{tag}{h}")
                    nc.vector.tensor_copy(wr[:], wt[:])
                    lst.append(wr)

                wqr, wkr = [], []
                for h in range(8):
                    load_w(wq, h, wqr, "wqr")

                def load_x(ib):
                    # plain fp16 DMAs on sync (SWDGE casting DMAs are slow),
                    # explicit DVE upcasts to fp32r
                    xs = []
                    for h in range(8):
                        xt = xst.tile([128, 512], FP16, tag="xst", name="xst")
                        nc.sync.dma_start(
                            xt[:], xq[h * 128:(h + 1) * 128,
                                      ib * 512:(ib + 1) * 512])
                        xr = xpool.tile([128, 512], FP32R, tag="xr", name="xr")
                        nc.vector.tensor_copy(xr[:], xt[:])
                        xs.append(xr)
                    return xs

                grows = [None] * 4  # (g1m_row fp16, g_row fp16) per i-block

                def emit_gate(ib):
                    # z = Qw @ Wv (dup pair cols so both outputs land on
                    # partition 0); 1-g = sigmoid(-z-bv), g = sigmoid(z+bv)
                    pzc = zpool.tile([2, 512], FP32, tag="zr", name="zr")
                    for a in range(4):
                        nc.tensor.matmul(
                            pzc[:], misc_r[:, 8 + 2 * a:10 + 2 * a],
                            qwt[a][:, ib * 512:(ib + 1) * 512],
                            start=(a == 0), stop=(a == 3))
                    g1 = gtmp.tile([1, 512], FP16, tag="g1", name="g1")
                    nc.scalar.activation(g1[:], pzc[0:1, :], AF.Sigmoid,
                                         scale=-1.0, bias=misc_sb[0:1, 17:18])
                    g2 = gtmp.tile([1, 512], FP16, tag="g2", name="g2")
                    nc.scalar.activation(g2[:], pzc[0:1, :], AF.Sigmoid,
                                         bias=misc_sb[0:1, 16:17])
                    grows[ib] = (g1, g2)

                def emit_bcast(ib):
                    # local planes for i-block ib via rank-1 ones matmuls
                    g1, g2 = grows[ib]
                    sl = slice(ib * 512, (ib + 1) * 512)
                    pb = bps.tile([128, 512], FP32, tag="pb", name="pb")
                    nc.tensor.matmul(pb[:], ones_h[:], g1[:],
                                     start=True, stop=True)
                    nc.vector.tensor_copy(g1m_bc[:, sl], pb[:])
                    pb2 = bps.tile([128, 512], FP32, tag="pb", name="pb")
                    nc.tensor.matmul(pb2[:], ones_h[:], g2[:],
                                     start=True, stop=True)
                    nc.scalar.copy(g_bc[:, sl], pb2[:])

                def emit_exchange(half):
                    # ship qwt[:, half*1024:(half+1)*1024]; gate rows ride
                    # AG2 only (its doorbell is gated by AG1 completion
                    # anyway, while AG1's must fire as early as possible)
                    sl = slice(half * 1024, (half + 1) * 1024)
                    for a in range(4):
                        xc = xch.tile([128, 1024], FP16, tag="xc", name="xc")
                        nc.vector.tensor_copy(xc[:], qwt[a][:, sl])
                        nc.gpsimd.dma_start(agin[half][a * 128:(a + 1) * 128, :],
                                            xc[:])
                    if half == 1:
                        for ib in range(4):
                            nc.gpsimd.dma_start(
                                agin[1][512 + ib // 2:513 + ib // 2,
                                        (ib % 2) * 512:(ib % 2 + 1) * 512],
                                grows[ib][0][:])
                    nc.gpsimd.collective_compute(
                        "AllGather", ALU.bypass,
                        replica_groups=[[0, 1], [2, 3], [4, 5], [6, 7]],
                        ins=[agin[half][:].opt()], outs=[agout[half][:].opt()],
                    )

                # ---- Q pass (local i-blocks); AG1 fires right after the
                # first two blocks' qwt exists. Gate deferred by one block so
                # its z-matmuls never head-block ready projection matmuls.
                xs_next = load_x(0)
                for ib in range(4):
                    xs = xs_next
                    if ib == 0:
                        xs_next = load_x(1)
                        for h in range(8):
                            load_w(wk, h, wkr, "wkr")
                    else:
                        xs_next = load_x(ib + 1) if ib < 3 else None
                    for a in range(4):
                        pq = ppool.tile([128, 512], FP32, tag="ps", name="ps")
                        for h in range(8):
                            nc.tensor.matmul(pq[:], wqr[h][:, a * 128:(a + 1) * 128],
                                             xs[h][:], start=(h == 0), stop=(h == 7))
                        nc.scalar.activation(qwt[a][:, ib * 512:(ib + 1) * 512],
                                             pq[:], AF.Identity,
                                             bias=misc_sb[:, a:a + 1])
                    if ib == 2:
                        emit_exchange(0)
                    if ib >= 1:
                        emit_gate(ib - 1)

                # ---- K pass (x re-read from HBM; the sync queue is quiet)
                xs_next = load_x(0)
                for ib in range(4):
                    xs = xs_next
                    xs_next = load_x(ib + 1) if ib < 3 else None
                    for a in range(4):
                        pk = ppool.tile([128, 512], FP32, tag="ps", name="ps")
                        for h in range(8):
                            nc.tensor.matmul(pk[:], wkr[h][:, a * 128:(a + 1) * 128],
                                             xs[h][:], start=(h == 0), stop=(h == 7))
                        nc.scalar.activation(kwt[a][:, ib * 512:(ib + 1) * 512],
                                             pk[:], AF.Identity,
                                             bias=misc_sb[:, 4 + a:5 + a])
                    if ib == 0:
                        emit_gate(3)
                        emit_exchange(1)
                    if ib >= 1:
                        emit_bcast(ib - 1)
                emit_bcast(3)

            # ---- score loop over 16 column tiles (output stays transposed).
            # Exp data lives in [128, 1024] quarters so lo / ch2 / ch3
            # chunks have independent lifetimes.
            with (
                tc.tile_pool(name="equart", bufs=30) as epool,
                tc.tile_pool(name="dsum", bufs=20) as dpool,
                tc.tile_pool(name="diag", bufs=2) as dzpool,
                tc.tile_pool(name="scoreps", bufs=3, space="PSUM") as sps,
                tc.tile_pool(name="bcps2", bufs=1, space="PSUM") as bps2,
            ):
                exps = {}
                dsums = [None] * 16

                def score_unit(t, ch):
                    # one 1024-wide chunk of score tile t: 8 matmuls + exp
                    if dsums[t] is None:
                        dsums[t] = dpool.tile([128, 4], FP32, tag="ds",
                                              name="ds")
                    ds = dsums[t]
                    et = epool.tile([128, 1024], BF16, tag="e", name="e")
                    exps[(t, ch)] = et
                    ps = sps.tile([128, 1024], FP32, tag="sc", name="sc")
                    for sub in range(2):
                        o = ch * 1024 + sub * 512
                        for a in range(4):
                            nc.tensor.matmul(ps[:, sub * 512:(sub + 1) * 512],
                                             kwt[a][:, t * 128:(t + 1) * 128],
                                             qwt[a][:, o:o + 512],
                                             start=(a == 0), stop=(a == 3))
                    if ch == (t * 128) // 1024:
                        off = t * 128 - ch * 1024
                        nc.vector.tensor_add(ps[:, off:off + 128],
                                             ps[:, off:off + 128], dneg[:])
                    nc.scalar.activation(et[:], ps[:], AF.Exp,
                                         accum_out=ds[:, ch:ch + 1])

                def readback(half):
                    # indirect row-gather of the peer qwt quarter (rank-
                    # ordered AG output; per-core host index vector), then
                    # DVE upcast. Safe on the DVE queue here: everything
                    # emitted after it is AG-gated anyway.
                    sl = slice(NSH + half * 1024, NSH + (half + 1) * 1024)
                    for a in range(4):
                        gt = xch.tile([128, 1024], FP16, tag="gt", name="gt")
                        nc.gpsimd.indirect_dma_start(
                            out=gt[:], out_offset=None,
                            in_=agout[half][:],
                            in_offset=bass.IndirectOffsetOnAxis(
                                ap=idx_sb[:, 4 * half + a:4 * half + a + 1],
                                axis=0),
                        )
                        nc.vector.tensor_copy(qwt[a][:, sl], gt[:])

                def remote_g1m():
                    # peer g1 rows -> g1m planes for blocks 4-7
                    grem = grp.tile([2, 1024], FP16, tag="gr", name="gr")
                    nc.gpsimd.indirect_dma_start(
                        out=grem[:], out_offset=None,
                        in_=agout[1][:],
                        in_offset=bass.IndirectOffsetOnAxis(
                            ap=idx_sb[0:2, 8:9], axis=0),
                    )
                    # matmul operands must start at partition 0/32/64:
                    # shuffle row 1 down via a tiny SBUF->SBUF DMA
                    grem2 = grp.tile([1, 1024], FP16, tag="gr2", name="gr2")
                    nc.sync.dma_start(grem2[:], grem[1:2, :])
                    rows = [grem[0:1, 0:512], grem[0:1, 512:1024],
                            grem2[0:1, 0:512], grem2[0:1, 512:1024]]
                    for blk in range(4):
                        sl = slice(NSH + blk * 512, NSH + (blk + 1) * 512)
                        pb = bps2.tile([128, 512], FP32, tag="pb", name="pb")
                        nc.tensor.matmul(pb[:], ones_h[:], rows[blk],
                                         start=True, stop=True)
                        nc.vector.tensor_copy(g1m_bc[:, sl], pb[:])

                def finalize(t, last=False):
                    ds = dsums[t]
                    rcol = dpool.tile([128, 1], FP32, tag="r", name="r")
                    nc.vector.tensor_reduce(rcol[:], ds[:], axis=AX.X,
                                            op=ALU.add)
                    nc.vector.reciprocal(rcol[:], rcol[:])
                    eng, eng2 = ((nc.sync, nc.gpsimd) if t % 2 == 1
                                 else (nc.gpsimd, nc.sync))
                    dch = (t * 128) // 1024
                    for ch in range(4):
                        et = exps[(t, ch)]
                        nc.vector.scalar_tensor_tensor(
                            et[:], et[:], rcol[:],
                            g1m_bc[:, ch * 1024:(ch + 1) * 1024],
                            op0=ALU.mult, op1=ALU.mult)
                        if ch == dch:
                            dz = dzpool.tile([128, 128], BF16, tag="dz",
                                             name="dz")
                            nc.vector.tensor_mul(
                                dz[:], identb[:],
                                g_bc[:, t * 128:(t + 1) * 128])
                            off = t * 128 - ch * 1024
                            nc.vector.tensor_add(et[:, off:off + 128],
                                                 et[:, off:off + 128], dz[:])
                        e = eng if ch % 2 == 0 else eng2
                        e.dma_start(out[t * 128:(t + 1) * 128,
                                        ch * 1024:(ch + 1) * 1024], et[:])

                # arrival-ordered emission (in-order PE queue must never
                # head-block on a collective):
                for t in range(8):
                    score_unit(t, 0)
                    score_unit(t, 1)
                readback(0)
                for t in range(8):
                    score_unit(t, 2)
                readback(1)
                remote_g1m()
                for k in range(8):
                    score_unit(k, 3)
                    finalize(k)
                    tf = 8 + k
                    for ch in range(4):
                        score_unit(tf, ch)
                    finalize(tf, last=(k == 7))
    nc.compile()
    return nc


def kernel(x, Wq, bq, Wk, bk, Wv, bv, _trace=False, _tmpdir=None):
    x = np.asarray(x, dtype=np.float32)
    if "nc" not in _CACHE:
        _CACHE["nc"] = _build()
    nc = _CACHE["nc"]

    bv_f = np.float32(np.asarray(bv).reshape(())[()])
    eye_np = np.eye(128, dtype=np.float32)
    misc = np.zeros((128, 18), dtype=np.float32)
    misc[:, 0:4] = np.asarray(bq, np.float32).reshape(4, 128).T
    misc[:, 4:8] = np.asarray(bk, np.float32).reshape(4, 128).T
    wv_c = np.asarray(Wv, np.float32).reshape(4, 128).T
    misc[:, 8:16:2] = wv_c
    misc[:, 9:16:2] = wv_c
    misc[:, 16] = bv_f
    misc[:, 17] = -bv_f
    wq_np = np.ascontiguousarray(np.asarray(Wq, np.float32).astype(np.float16))
    wk_np = np.ascontiguousarray(np.asarray(Wk, np.float32).astype(np.float16))

    in_maps = []
    for c in range(8):
        b, h = c // 2, c % 2
        # local rows only: perm-i [0,2048) = orig rows [h*2048,(h+1)*2048)
        xqc = np.ascontiguousarray(
            x[b].T[:, h * NSH:(h + 1) * NSH].astype(np.float16))
        idx = np.zeros((128, 9), dtype=np.int32)
        for a in range(4):
            idx[:, a] = (1 - h) * 512 + a * 128 + np.arange(128)
            idx[:, 4 + a] = (1 - h) * AGR2 + a * 128 + np.arange(128)
        idx[0:2, 8] = (1 - h) * AGR2 + 512 + np.arange(2)
        in_maps.append({"xq": xqc, "wq": wq_np, "wk": wk_np, "misc": misc,
                        "eye": eye_np, "idx": idx})

    res = run_bass_kernel_spmd(nc, in_maps, list(range(8)), trace=_trace,
                               tmpdir=_tmpdir)

    outp = np.empty((B, N, N), dtype=np.float32)
    for c in range(8):
        b, h = c // 2, c % 2
        O = np.asarray(res.results[c]["out"]).astype(np.float32).T  # (i_perm, j)
        js = slice(h * NSH, (h + 1) * NSH)
        outp[b, h * NSH:(h + 1) * NSH, js] = O[:NSH]
        outp[b, (1 - h) * NSH:(2 - h) * NSH, js] = O[NSH:]
    if _trace:
        return outp, res
    return outp
